# revision 10
# baseline (speedup 1.0000x reference)
"""Trainium2 Bass kernel for the augmented-ODE-RHS (primal + 4 JVPs) problem.

Math (per sample; w=omega, v=omega_dot, K=(k0..k3), aug pairs (a_p, b_p)):
    mM = k0*w + k1*v            M = 10 - mM        A = 1/M
    mD = k2*w + k3*v            E = mD - 1 (= -D)
    u  = 0.2*w + v
    g  = 0.02 - 4*w + E*u       P = A*g
    f2 = P - 0.2*v
    out[0] = v, out[1] = f2
JVP p (tangent (a_p, b_p, e_p)) collapses to a per-sample linear form:
    alpha = -4A + 0.2*A*E + (A*u)*k2 + (A*P)*k0
    beta  = A*E - 0.2 + (A*u)*k3 + (A*P)*k1
    gamma_p in (A*P*w, A*P*v, A*u*w, A*u*v)
    out[2+2p] = b_p,  out[3+2p] = alpha*a_p + beta*b_p + gamma_p

Sharding: pure data parallel over the batch across 8 NeuronCores. Each core
gets R = 128*CHUNKS*N rows (inputs zero-padded up to 8R). Per core, rows are
laid out so SBUF partition j owns a contiguous slab of rows -> every DMA is
128 fully-contiguous multi-KB segments.

Engine split per chunk (fp32): DVE does the tensor*tensor products (30N),
GPSIMD the pure adds (14N), ACT the affine/copies (9N), HWDGE the DMAs.
"""

import json

import numpy as np

N_CORES = 8
P = 128
CHUNKS = 10

_CACHE: dict = {}


def _fix_bir_json(raw: bytes) -> bytes:
    """Walrus in this container encodes at most ONE sem-wait and ONE sem-update
    per instruction. Tile attaches several. Split the extras onto single-wait /
    single-update EventSemaphore instructions on the same engine, placed just
    before (waits) / after (updates) the original — identical sync semantics."""
    m = json.loads(raw)
    ctr = 0
    for fn in m["functions"]:
        for blk in fn["blocks"]:
            out = []
            for ins in blk["instructions"]:
                si = ins.get("sync_info")
                pend_updates = []
                if si:
                    waits = si.get("on_wait") or []
                    if len(waits) > 1:
                        for w in waits[:-1]:
                            ctr += 1
                            ev = {
                                "engine": ins["engine"], "ins": [], "outs": [],
                                "name": f"xw-{ctr}",
                                "opcode": "EventSemaphore",
                                "sync_info": {"on_update": [], "on_wait": [w]},
                            }
                            if "debug" in ins:
                                ev["debug"] = ins["debug"]
                            out.append(ev)
                        si["on_wait"] = [waits[-1]]
                    ups = si.get("on_update") or []
                    if len(ups) > 1:
                        assert ins.get("opcode") != "DMACopy", \
                            "DMACopy with >1 sem updates cannot be split"
                        si["on_update"] = [ups[0]]
                        pend_updates = ups[1:]
                out.append(ins)
                for u in pend_updates:
                    ctr += 1
                    ev = {
                        "engine": ins["engine"], "ins": [], "outs": [],
                        "name": f"xu-{ctr}",
                        "opcode": "EventSemaphore",
                        "sync_info": {"on_update": [u], "on_wait": []},
                    }
                    if "debug" in ins:
                        ev["debug"] = ins["debug"]
                    out.append(ev)
            blk["instructions"] = out
    return json.dumps(m).encode()


def _build(R: int, N: int, reps: int = 1):
    import concourse.bass as bass
    import concourse.tile as tile
    import concourse.mybir as mybir

    F32 = mybir.dt.float32
    mul = mybir.AluOpType.mult
    add = mybir.AluOpType.add
    Copy = mybir.ActivationFunctionType.Copy

    nc = bass.Bass("TRN2")

    state_d = nc.dram_tensor("state", [R, 10], F32, kind="ExternalInput")
    k_d = nc.dram_tensor("K", [R, 4], F32, kind="ExternalInput")
    out_d = nc.dram_tensor("out", [R, 10], F32, kind="ExternalOutput")

    sv = state_d[:].rearrange("(p n) m -> p (n m)", p=P)
    kv = k_d[:].rearrange("(p n) m -> p (n m)", p=P)
    ov = out_d[:].rearrange("(p n) m -> p (n m)", p=P)

    with tile.TileContext(nc) as tc:
        with (
            tc.tile_pool(name="io", bufs=2) as io,
            tc.tile_pool(name="tmp", bufs=1) as tp,
            tc.tile_pool(name="tmp2", bufs=2) as tp2,
        ):
            for c in [c for _ in range(reps) for c in range(CHUNKS)]:
                S_t = io.tile([P, 10 * N], F32, tag="S")
                K_t = io.tile([P, 4 * N], F32, tag="K")
                O_t = io.tile([P, 10 * N], F32, tag="O")
                nc.sync.dma_start(S_t[:], sv[:, c * 10 * N:(c + 1) * 10 * N])
                nc.sync.dma_start(K_t[:], kv[:, c * 4 * N:(c + 1) * 4 * N])

                S5 = S_t[:].rearrange("p (n c two) -> p n c two", two=2, c=5)
                O5 = O_t[:].rearrange("p (n c two) -> p n c two", two=2, c=5)
                Kt22 = K_t[:].rearrange("p (n c two) -> p n c two", two=2, c=2)
                Kt4 = K_t[:].rearrange("p (n f) -> p n f", f=4)

                w3 = S5[:, :, 0:1, 0]     # [P,N,1]
                v3 = S5[:, :, 0:1, 1]
                wv3 = S5[:, :, 0, :]      # [P,N,2]
                a4 = S5[:, :, 1:5, 0]     # [P,N,4]
                b4 = S5[:, :, 1:5, 1]
                k02 = Kt22[:, :, :, 0]    # (k0,k2)
                k13 = Kt22[:, :, :, 1]    # (k1,k3)
                k01 = Kt4[:, :, 0:2]
                k23 = Kt4[:, :, 2:4]

                X_t = tp.tile([P, 2 * N], F32, tag="X")
                Y_t = tp.tile([P, 2 * N], F32, tag="Y")
                MD_t = tp2.tile([P, 2 * N], F32, tag="MD")
                Mb_t = tp.tile([P, N], F32, tag="Mb")
                ln_t = tp.tile([P, N], F32, tag="ln")
                A_t = tp2.tile([P, N], F32, tag="A")
                E_t = tp2.tile([P, N], F32, tag="E")
                PU_t = tp2.tile([P, 2 * N], F32, tag="PU")
                T3_t = tp.tile([P, N], F32, tag="T3")
                h_t = tp.tile([P, N], F32, tag="h")
                AE_t = tp.tile([P, N], F32, tag="AE")
                CMU_t = tp2.tile([P, 2 * N], F32, tag="CMU")
                ca0_t = tp.tile([P, N], F32, tag="ca0")
                CAB_t = tp2.tile([P, 2 * N], F32, tag="CAB")
                T4_t = tp.tile([P, 2 * N], F32, tag="T4")
                T5_t = tp.tile([P, 2 * N], F32, tag="T5")
                T6_t = tp.tile([P, 2 * N], F32, tag="T6")
                AB_t = tp2.tile([P, 2 * N], F32, tag="AB")
                T7a_t = tp.tile([P, 4 * N], F32, tag="T7a")
                T7b_t = tp.tile([P, 4 * N], F32, tag="T7b")
                T8_t = tp.tile([P, 4 * N], F32, tag="T8")
                G_t = tp.tile([P, 4 * N], F32, tag="G")

                X2 = X_t[:].rearrange("p (n two) -> p n two", two=2)
                Y2 = Y_t[:].rearrange("p (n two) -> p n two", two=2)
                MD2 = MD_t[:].rearrange("p (n two) -> p n two", two=2)
                PU2 = PU_t[:].rearrange("p (n two) -> p n two", two=2)
                CMU2 = CMU_t[:].rearrange("p (n two) -> p n two", two=2)
                CAB2 = CAB_t[:].rearrange("p (n two) -> p n two", two=2)
                AB2 = AB_t[:].rearrange("p (n two) -> p n two", two=2)
                T7a2 = T7a_t[:].rearrange("p (n f) -> p n f", f=4)
                T7b2 = T7b_t[:].rearrange("p (n f) -> p n f", f=4)
                T82 = T8_t[:].rearrange("p (n f) -> p n f", f=4)
                G2 = G_t[:].rearrange("p (n f) -> p n f", f=4)

                A3 = A_t[:].unsqueeze(2)
                E3 = E_t[:].unsqueeze(2)

                # X=(k0,k2)*w ; Y=(k1,k3)*v ; MD=X+Y=(mM,mD)
                nc.vector.tensor_mul(X2, k02, w3.broadcast_to([P, N, 2]))
                nc.vector.tensor_mul(Y2, k13, v3.broadcast_to([P, N, 2]))
                nc.gpsimd.tensor_add(MD_t[:], X_t[:], Y_t[:])

                # Mb = 10 - mM ; E = mD - 1 ; A = 1/Mb
                nc.scalar.activation(Mb_t[:].unsqueeze(2), MD2[:, :, 0:1], Copy,
                                     bias=10.0, scale=-1.0)
                nc.scalar.activation(E3, MD2[:, :, 1:2], Copy,
                                     bias=-1.0, scale=1.0)
                # A = 1/Mb via exp(-ln(Mb)) on ACT (Mb > 0 always: Mb = 10 - mM)
                nc.scalar.activation(ln_t[:], Mb_t[:],
                                     mybir.ActivationFunctionType.Ln)
                nc.scalar.activation(A_t[:], ln_t[:],
                                     mybir.ActivationFunctionType.Exp, scale=-1.0)

                # u = 0.2w + v ; T3 = E*u ; h = -4w + T3 ; P = (h+0.02)*A
                nc.vector.scalar_tensor_tensor(PU2[:, :, 0:1], w3, 0.2, v3, mul, add)
                nc.vector.tensor_mul(T3_t[:].unsqueeze(2), E3, PU2[:, :, 0:1])
                nc.vector.scalar_tensor_tensor(h_t[:].unsqueeze(2), w3, -4.0,
                                               T3_t[:].unsqueeze(2), mul, add)
                nc.vector.scalar_tensor_tensor(PU2[:, :, 1:2], h_t[:].unsqueeze(2),
                                               0.02, A3, add, mul)

                # AE = A*E ; (c_u,c_m) = A*(u,P)
                nc.vector.tensor_mul(AE_t[:].unsqueeze(2), A3, E3)
                nc.vector.tensor_mul(CMU2, A3.broadcast_to([P, N, 2]), PU2)

                # c_a = 0.2AE - 4A ; c_b = AE - 0.2
                nc.scalar.activation(ca0_t[:].unsqueeze(2), A3, Copy, scale=-4.0)
                nc.vector.scalar_tensor_tensor(CAB2[:, :, 0:1], AE_t[:].unsqueeze(2),
                                               0.2, ca0_t[:].unsqueeze(2), mul, add)
                nc.scalar.activation(CAB2[:, :, 1:2], AE_t[:].unsqueeze(2), Copy,
                                     bias=-0.2, scale=1.0)

                c_u_bc2 = CMU2[:, :, 0:1].broadcast_to([P, N, 2])
                c_m_bc2 = CMU2[:, :, 1:2].broadcast_to([P, N, 2])

                # (alpha,beta) = (c_a,c_b) + c_u*(k2,k3) + c_m*(k0,k1)
                nc.vector.tensor_mul(
                    T4_t[:].rearrange("p (n two) -> p n two", two=2), c_u_bc2, k23)
                nc.vector.tensor_mul(
                    T5_t[:].rearrange("p (n two) -> p n two", two=2), c_m_bc2, k01)
                nc.gpsimd.tensor_add(T6_t[:], T4_t[:], T5_t[:])
                nc.gpsimd.tensor_add(AB_t[:], T6_t[:], CAB_t[:])

                # f2 = P - 0.2v -> out col 1
                nc.vector.scalar_tensor_tensor(O5[:, :, 0:1, 1], v3, -0.2,
                                               PU2[:, :, 1:2], mul, add)

                # df2_p = alpha*a_p + beta*b_p + gamma_p -> out cols 3,5,7,9
                nc.vector.tensor_mul(T7a2, AB2[:, :, 0:1].broadcast_to([P, N, 4]), a4)
                nc.vector.tensor_mul(T7b2, AB2[:, :, 1:2].broadcast_to([P, N, 4]), b4)
                nc.gpsimd.tensor_add(T8_t[:], T7a_t[:], T7b_t[:])
                nc.vector.tensor_mul(G2[:, :, 0:2], c_m_bc2, wv3)
                nc.vector.tensor_mul(G2[:, :, 2:4], c_u_bc2, wv3)
                nc.gpsimd.tensor_add(O5[:, :, 1:5, 1], T82, G2)

                # out even cols = state odd cols
                nc.scalar.activation(O5[:, :, :, 0], S5[:, :, :, 1], Copy)

                nc.sync.dma_start(ov[:, c * 10 * N:(c + 1) * 10 * N], O_t[:])

    orig = nc.to_json_bytes
    nc.to_json_bytes = lambda: _fix_bir_json(orig())
    return nc


def _get_program(B: int, reps: int = 1):
    key = (B, reps)
    if key not in _CACHE:
        N = -(-B // (N_CORES * P * CHUNKS))  # ceil
        R = P * CHUNKS * N
        _CACHE[key] = (_build(R, N, reps), R)
    return _CACHE[key]


def _run(state: np.ndarray, K: np.ndarray, trace: bool = False, reps: int = 1):
    from concourse import bass_utils

    B = state.shape[0]
    nc, R = _get_program(B, reps)
    BP = N_CORES * R

    state_p = np.zeros((BP, 10), dtype=np.float32)
    state_p[:B] = state
    k_p = np.zeros((BP, 4), dtype=np.float32)
    k_p[:B] = K

    in_maps = [
        {"state": state_p[i * R:(i + 1) * R], "K": k_p[i * R:(i + 1) * R]}
        for i in range(N_CORES)
    ]
    res = bass_utils.run_bass_kernel_spmd(
        nc, in_maps, core_ids=list(range(N_CORES)), trace=trace
    )
    out = np.concatenate([r["out"] for r in res.results], axis=0)[:B]
    return out, res


def kernel(t, state, K):
    state = np.ascontiguousarray(np.asarray(state), dtype=np.float32)
    K = np.ascontiguousarray(np.asarray(K), dtype=np.float32)
    out, _ = _run(state, K, trace=False)
    return out


# revision 11
# speedup vs baseline: 7.5241x; 7.5241x over previous
"""Trainium2 Bass kernel for the augmented-ODE-RHS (primal + 4 JVPs) problem.

Math (per sample; w=omega, v=omega_dot, K=(k0..k3), aug pairs (a_p, b_p)):
    mM = k0*w + k1*v            M = 10 - mM        A = 1/M
    mD = k2*w + k3*v            E = mD - 1 (= -D)
    u  = 0.2*w + v
    g  = 0.02 - 4*w + E*u       P = A*g
    f2 = P - 0.2*v
    out[0] = v, out[1] = f2
JVP p (tangent (a_p, b_p, e_p)) collapses to a per-sample linear form:
    alpha = -4A + 0.2*A*E + (A*u)*k2 + (A*P)*k0
    beta  = A*E - 0.2 + (A*u)*k3 + (A*P)*k1
    gamma_p in (A*P*w, A*P*v, A*u*w, A*u*v)
    out[2+2p] = b_p,  out[3+2p] = alpha*a_p + beta*b_p + gamma_p

Sharding: pure data parallel over the batch across 8 NeuronCores. Each core
gets R = 128*CHUNKS*N rows (inputs zero-padded up to 8R). Per core, rows are
laid out so SBUF partition j owns a contiguous slab of rows -> every DMA is
128 fully-contiguous multi-KB segments.

Engine split per chunk (fp32): DVE does the tensor*tensor products (30N),
GPSIMD the pure adds (14N), ACT the affine/copies (9N), HWDGE the DMAs.
"""

import json

import numpy as np

N_CORES = 8
P = 128
CHUNKS = 10

_CACHE: dict = {}


def _fix_bir_json(raw: bytes) -> bytes:
    """Walrus in this container encodes at most ONE sem-wait and ONE sem-update
    per instruction. Tile attaches several. Split the extras onto single-wait /
    single-update EventSemaphore instructions on the same engine, placed just
    before (waits) / after (updates) the original — identical sync semantics."""
    m = json.loads(raw)
    ctr = 0
    for fn in m["functions"]:
        for blk in fn["blocks"]:
            out = []
            for ins in blk["instructions"]:
                si = ins.get("sync_info")
                pend_updates = []
                if si:
                    waits = si.get("on_wait") or []
                    if len(waits) > 1:
                        for w in waits[:-1]:
                            ctr += 1
                            ev = {
                                "engine": ins["engine"], "ins": [], "outs": [],
                                "name": f"xw-{ctr}",
                                "opcode": "EventSemaphore",
                                "sync_info": {"on_update": [], "on_wait": [w]},
                            }
                            if "debug" in ins:
                                ev["debug"] = ins["debug"]
                            out.append(ev)
                        si["on_wait"] = [waits[-1]]
                    ups = si.get("on_update") or []
                    if len(ups) > 1:
                        assert ins.get("opcode") != "DMACopy", \
                            "DMACopy with >1 sem updates cannot be split"
                        si["on_update"] = [ups[0]]
                        pend_updates = ups[1:]
                out.append(ins)
                for u in pend_updates:
                    ctr += 1
                    ev = {
                        "engine": ins["engine"], "ins": [], "outs": [],
                        "name": f"xu-{ctr}",
                        "opcode": "EventSemaphore",
                        "sync_info": {"on_update": [u], "on_wait": []},
                    }
                    if "debug" in ins:
                        ev["debug"] = ins["debug"]
                    out.append(ev)
            blk["instructions"] = out
    return json.dumps(m).encode()


def _build(R: int, N: int, reps: int = 1):
    import concourse.bass as bass
    import concourse.tile as tile
    import concourse.mybir as mybir

    F32 = mybir.dt.float32
    mul = mybir.AluOpType.mult
    add = mybir.AluOpType.add
    Copy = mybir.ActivationFunctionType.Copy

    nc = bass.Bass("TRN2")

    state_d = nc.dram_tensor("state", [R, 10], F32, kind="ExternalInput")
    k_d = nc.dram_tensor("K", [R, 4], F32, kind="ExternalInput")
    out_d = nc.dram_tensor("out", [R, 10], F32, kind="ExternalOutput")

    sv = state_d[:].rearrange("(p n) m -> p (n m)", p=P)
    kv = k_d[:].rearrange("(p n) m -> p (n m)", p=P)
    ov = out_d[:].rearrange("(p n) m -> p (n m)", p=P)

    with tile.TileContext(nc) as tc:
        with (
            tc.tile_pool(name="io", bufs=2) as io,
            tc.tile_pool(name="tmp", bufs=1) as tp,
            tc.tile_pool(name="tmp2", bufs=2) as tp2,
        ):
            for c in [c for _ in range(reps) for c in range(CHUNKS)]:
                S_t = io.tile([P, 10 * N], F32, tag="S")
                K_t = io.tile([P, 4 * N], F32, tag="K")
                O_t = io.tile([P, 10 * N], F32, tag="O")
                nc.sync.dma_start(S_t[:], sv[:, c * 10 * N:(c + 1) * 10 * N])
                nc.sync.dma_start(K_t[:], kv[:, c * 4 * N:(c + 1) * 4 * N])

                S5 = S_t[:].rearrange("p (n c two) -> p n c two", two=2, c=5)
                O5 = O_t[:].rearrange("p (n c two) -> p n c two", two=2, c=5)
                Kt22 = K_t[:].rearrange("p (n c two) -> p n c two", two=2, c=2)
                Kt4 = K_t[:].rearrange("p (n f) -> p n f", f=4)

                w3 = S5[:, :, 0:1, 0]     # [P,N,1]
                v3 = S5[:, :, 0:1, 1]
                wv3 = S5[:, :, 0, :]      # [P,N,2]
                a4 = S5[:, :, 1:5, 0]     # [P,N,4]
                b4 = S5[:, :, 1:5, 1]
                k02 = Kt22[:, :, :, 0]    # (k0,k2)
                k13 = Kt22[:, :, :, 1]    # (k1,k3)
                k01 = Kt4[:, :, 0:2]
                k23 = Kt4[:, :, 2:4]

                X_t = tp.tile([P, 2 * N], F32, tag="X")
                Y_t = tp.tile([P, 2 * N], F32, tag="Y")
                MD_t = tp2.tile([P, 2 * N], F32, tag="MD")
                Mb_t = tp.tile([P, N], F32, tag="Mb")
                ln_t = tp.tile([P, N], F32, tag="ln")
                A_t = tp2.tile([P, N], F32, tag="A")
                E_t = tp2.tile([P, N], F32, tag="E")
                PU_t = tp2.tile([P, 2 * N], F32, tag="PU")
                T3_t = tp.tile([P, N], F32, tag="T3")
                h_t = tp.tile([P, N], F32, tag="h")
                AE_t = tp.tile([P, N], F32, tag="AE")
                CMU_t = tp2.tile([P, 2 * N], F32, tag="CMU")
                ca0_t = tp.tile([P, N], F32, tag="ca0")
                CAB_t = tp2.tile([P, 2 * N], F32, tag="CAB")
                T4_t = tp.tile([P, 2 * N], F32, tag="T4")
                T5_t = tp.tile([P, 2 * N], F32, tag="T5")
                T6_t = tp.tile([P, 2 * N], F32, tag="T6")
                AB_t = tp2.tile([P, 2 * N], F32, tag="AB")
                T7a_t = tp.tile([P, 4 * N], F32, tag="T7a")
                T7b_t = tp.tile([P, 4 * N], F32, tag="T7b")
                T8_t = tp.tile([P, 4 * N], F32, tag="T8")
                G_t = tp.tile([P, 4 * N], F32, tag="G")

                X2 = X_t[:].rearrange("p (n two) -> p n two", two=2)
                Y2 = Y_t[:].rearrange("p (n two) -> p n two", two=2)
                MD2 = MD_t[:].rearrange("p (n two) -> p n two", two=2)
                PU2 = PU_t[:].rearrange("p (n two) -> p n two", two=2)
                CMU2 = CMU_t[:].rearrange("p (n two) -> p n two", two=2)
                CAB2 = CAB_t[:].rearrange("p (n two) -> p n two", two=2)
                AB2 = AB_t[:].rearrange("p (n two) -> p n two", two=2)
                T7a2 = T7a_t[:].rearrange("p (n f) -> p n f", f=4)
                T7b2 = T7b_t[:].rearrange("p (n f) -> p n f", f=4)
                T82 = T8_t[:].rearrange("p (n f) -> p n f", f=4)
                G2 = G_t[:].rearrange("p (n f) -> p n f", f=4)

                A3 = A_t[:].unsqueeze(2)
                E3 = E_t[:].unsqueeze(2)

                # X=(k0,k2)*w ; Y=(k1,k3)*v ; MD=X+Y=(mM,mD)
                nc.vector.tensor_mul(X2, k02, w3.broadcast_to([P, N, 2]))
                nc.vector.tensor_mul(Y2, k13, v3.broadcast_to([P, N, 2]))
                nc.gpsimd.tensor_add(MD_t[:], X_t[:], Y_t[:])

                # Mb = 10 - mM ; E = mD - 1 ; A = 1/Mb
                nc.scalar.activation(Mb_t[:].unsqueeze(2), MD2[:, :, 0:1], Copy,
                                     bias=10.0, scale=-1.0)
                nc.scalar.activation(E3, MD2[:, :, 1:2], Copy,
                                     bias=-1.0, scale=1.0)
                # A = 1/Mb via exp(-ln(Mb)) on ACT (Mb > 0 always: Mb = 10 - mM)
                nc.scalar.activation(ln_t[:], Mb_t[:],
                                     mybir.ActivationFunctionType.Ln)
                nc.scalar.activation(A_t[:], ln_t[:],
                                     mybir.ActivationFunctionType.Exp, scale=-1.0)

                # u = 0.2w + v ; T3 = E*u ; h = -4w + T3 ; P = (h+0.02)*A
                nc.vector.scalar_tensor_tensor(PU2[:, :, 0:1], w3, 0.2, v3, mul, add)
                nc.vector.tensor_mul(T3_t[:].unsqueeze(2), E3, PU2[:, :, 0:1])
                nc.vector.scalar_tensor_tensor(h_t[:].unsqueeze(2), w3, -4.0,
                                               T3_t[:].unsqueeze(2), mul, add)
                nc.vector.scalar_tensor_tensor(PU2[:, :, 1:2], h_t[:].unsqueeze(2),
                                               0.02, A3, add, mul)

                # AE = A*E ; (c_u,c_m) = A*(u,P)
                nc.vector.tensor_mul(AE_t[:].unsqueeze(2), A3, E3)
                nc.vector.tensor_mul(CMU2, A3.broadcast_to([P, N, 2]), PU2)

                # c_a = 0.2AE - 4A ; c_b = AE - 0.2
                nc.scalar.activation(ca0_t[:].unsqueeze(2), A3, Copy, scale=-4.0)
                nc.vector.scalar_tensor_tensor(CAB2[:, :, 0:1], AE_t[:].unsqueeze(2),
                                               0.2, ca0_t[:].unsqueeze(2), mul, add)
                nc.scalar.activation(CAB2[:, :, 1:2], AE_t[:].unsqueeze(2), Copy,
                                     bias=-0.2, scale=1.0)

                c_u_bc2 = CMU2[:, :, 0:1].broadcast_to([P, N, 2])
                c_m_bc2 = CMU2[:, :, 1:2].broadcast_to([P, N, 2])

                # (alpha,beta) = (c_a,c_b) + c_u*(k2,k3) + c_m*(k0,k1)
                nc.vector.tensor_mul(
                    T4_t[:].rearrange("p (n two) -> p n two", two=2), c_u_bc2, k23)
                nc.vector.tensor_mul(
                    T5_t[:].rearrange("p (n two) -> p n two", two=2), c_m_bc2, k01)
                nc.gpsimd.tensor_add(T6_t[:], T4_t[:], T5_t[:])
                nc.gpsimd.tensor_add(AB_t[:], T6_t[:], CAB_t[:])

                # f2 = P - 0.2v -> out col 1
                nc.vector.scalar_tensor_tensor(O5[:, :, 0:1, 1], v3, -0.2,
                                               PU2[:, :, 1:2], mul, add)

                # df2_p = alpha*a_p + beta*b_p + gamma_p -> out cols 3,5,7,9
                nc.vector.tensor_mul(T7a2, AB2[:, :, 0:1].broadcast_to([P, N, 4]), a4)
                nc.vector.tensor_mul(T7b2, AB2[:, :, 1:2].broadcast_to([P, N, 4]), b4)
                nc.gpsimd.tensor_add(T8_t[:], T7a_t[:], T7b_t[:])
                nc.vector.tensor_mul(G2[:, :, 0:2], c_m_bc2, wv3)
                nc.vector.tensor_mul(G2[:, :, 2:4], c_u_bc2, wv3)
                nc.gpsimd.tensor_add(O5[:, :, 1:5, 1], T82, G2)

                # out even cols = state odd cols
                nc.scalar.activation(O5[:, :, :, 0], S5[:, :, :, 1], Copy)

                nc.sync.dma_start(ov[:, c * 10 * N:(c + 1) * 10 * N], O_t[:])

    orig = nc.to_json_bytes
    nc.to_json_bytes = lambda: _fix_bir_json(orig())
    return nc


def _build2(R: int, N: int, reps: int = 1, chunks: int = 7):
    """v2: single-engine (DVE-only) minimal-instruction design.

    This platform charges a large fixed cost per engine instruction, so the
    kernel is organised as ~18 wide DVE ops per chunk, no cross-engine sync
    (outputs are computed in-place in the input state tile), HWDGE DMAs.
    """
    import concourse.bass as bass
    import concourse.tile as tile
    import concourse.mybir as mybir
    from concourse.ap import AP

    F32 = mybir.dt.float32
    mul = mybir.AluOpType.mult
    add = mybir.AluOpType.add
    sub = mybir.AluOpType.subtract

    nc = bass.Bass("TRN2")
    state_d = nc.dram_tensor("state", [R, 10], F32, kind="ExternalInput")
    k_d = nc.dram_tensor("K", [R, 4], F32, kind="ExternalInput")
    out_d = nc.dram_tensor("out", [R, 10], F32, kind="ExternalOutput")
    sv = state_d[:].rearrange("(p n) m -> p (n m)", p=P)
    kv = k_d[:].rearrange("(p n) m -> p (n m)", p=P)
    ov = out_d[:].rearrange("(p n) m -> p (n m)", p=P)

    def mkap(tile_ap, offset, dims):
        # dims: list of [step, count] free dims; partition dim taken from tile
        part = tile_ap.ap[0]
        return AP(tile_ap.tensor, offset, [list(part)] + [list(d) for d in dims])

    with tile.TileContext(nc) as tc:
        with (
            tc.tile_pool(name="io", bufs=2) as io,
            tc.tile_pool(name="tmp", bufs=1) as tp,
            tc.tile_pool(name="const", bufs=1) as cp,
        ):
            C2 = cp.tile([P, 2], F32)      # [10, 1]
            ones = cp.tile([P, 1], F32)
            nc.vector.memset(C2[:, 0:1], 10.0)
            nc.vector.memset(C2[:, 1:2], 1.0)
            nc.vector.memset(ones[:], 1.0)

            for c in [c for _ in range(reps) for c in range(chunks)]:
                S_t = io.tile([P, 10 * N], F32, tag="S")
                K_t = io.tile([P, 4 * N], F32, tag="K")
                nc.sync.dma_start(S_t[:], sv[:, c * 10 * N:(c + 1) * 10 * N])
                nc.sync.dma_start(K_t[:], kv[:, c * 4 * N:(c + 1) * 4 * N])

                SC = tp.tile([P, 20 * N], F32, tag="SC")
                ZZ = tp.tile([P, 10 * N], F32, tag="ZZ")
                U5_t = tp.tile([P, 5 * N], F32, tag="U5")
                DU5_t = tp.tile([P, 5 * N], F32, tag="DU5")
                H5_t = tp.tile([P, 5 * N], F32, tag="H5")
                MD_t = tp.tile([P, 2 * N], F32, tag="MD")
                A_t = tp.tile([P, N], F32, tag="A")
                P_t = tp.tile([P, N], F32, tag="P")
                cm_t = tp.tile([P, N], F32, tag="cm")

                S5 = S_t[:].rearrange("p (n c two) -> p n c two", two=2, c=5)
                evens = S5[:, :, :, 0]                    # [P,N,5] strides (10,2)
                odds = S5[:, :, :, 1]
                ev_rep = evens.unsqueeze(2).broadcast_to([P, N, 2, 5])
                od_rep = odds.unsqueeze(2).broadcast_to([P, N, 2, 5])
                Kt22 = K_t[:].rearrange("p (n c two) -> p n c two", two=2, c=2)
                K02 = Kt22[:, :, :, 0].unsqueeze(3).broadcast_to([P, N, 2, 5])
                K13 = Kt22[:, :, :, 1].unsqueeze(3).broadcast_to([P, N, 2, 5])

                E2v = SC[:, :10 * N].rearrange("p (n a c) -> p n a c", a=2, c=5)
                Rv = SC[:, 10 * N:].rearrange("p (n a c) -> p n a c", a=2, c=5)
                ZZv = ZZ[:].rearrange("p (n a c) -> p n a c", a=2, c=5)
                U5v = U5_t[:].rearrange("p (n c) -> p n c", c=5)
                DU5v = DU5_t[:].rearrange("p (n c) -> p n c", c=5)
                H5v = H5_t[:].rearrange("p (n c) -> p n c", c=5)
                MDv = MD_t[:].rearrange("p (n c) -> p n c", c=2)
                A3 = A_t[:].unsqueeze(2)                  # [P,N,1]
                P3 = P_t[:].unsqueeze(2)
                cm3 = cm_t[:].unsqueeze(2)

                # 1-3: ZZ[j2,c] = k_{2j2}*S[2c] + k_{2j2+1}*S[2c+1]
                nc.vector.tensor_mul(E2v, K02, ev_rep)
                nc.vector.tensor_mul(Rv, K13, od_rep)
                nc.vector.tensor_add(ZZv, E2v, Rv)
                # 4: extras — ZZ slots {1,2,8,9} += (w,v,w,v)
                zz_ex = mkap(ZZ[:], 1, [[10, N], [7, 2], [1, 2]])
                wv_rep = mkap(S_t[:], 0, [[10, N], [0, 2], [1, 2]])
                nc.vector.tensor_add(zz_ex, zz_ex, wv_rep)
                # 5: MD = [10,1] - [mM, mD]
                c2b = mkap(C2[:], 0, [[0, N], [1, 2]])
                zz0 = mkap(ZZ[:], 0, [[10, N], [5, 2]])
                nc.vector.tensor_tensor(MDv, c2b, zz0, sub)
                # 6: A = 1/M
                nc.vector.reciprocal(A_t[:], MDv[:, :, 0])
                # 7: U5 = 0.2*evens + odds
                nc.vector.scalar_tensor_tensor(U5v, evens, 0.2, odds, mul, add)
                # 8: DU5 = D * U5
                nc.vector.tensor_mul(DU5v, MDv[:, :, 1:2].broadcast_to([P, N, 5]), U5v)
                # 9: NDU = u * nD_p   (SC[0:4N])
                NDU = SC[:, :4 * N].rearrange("p (n c) -> p n c", c=4)
                nc.vector.tensor_mul(NDU, U5v[:, :, 0:1].broadcast_to([P, N, 4]),
                                     ZZv[:, :, 1, 1:5])
                # 10: H5 = -4*evens - DU5
                nc.vector.scalar_tensor_tensor(H5v, evens, -4.0, DU5v, mul, sub)
                # 11: DG4 = H5[1:5] + NDU   (SC[4N:8N])
                DG4 = SC[:, 4 * N:8 * N].rearrange("p (n c) -> p n c", c=4)
                nc.vector.tensor_add(DG4, H5v[:, :, 1:5], NDU)
                # 12: P = (H5[0] + 0.02) * A
                nc.vector.scalar_tensor_tensor(P3, H5v[:, :, 0:1], 0.02, A3, add, mul)
                # 13: cm = A * P
                nc.vector.tensor_mul(cm3, A3, P3)
                # 14: Q4 = A * DG4   (SC[8N:12N])
                Q4 = SC[:, 8 * N:12 * N].rearrange("p (n c) -> p n c", c=4)
                nc.vector.tensor_mul(Q4, A3.broadcast_to([P, N, 4]), DG4)
                # 15: R4 = cm * nM_p   (SC[12N:16N])
                R4 = SC[:, 12 * N:16 * N].rearrange("p (n c) -> p n c", c=4)
                nc.vector.tensor_mul(R4, cm3.broadcast_to([P, N, 4]),
                                     ZZv[:, :, 0, 1:5])
                # 16: S4 = Q4 + R4   (SC[16N:20N])
                S4 = SC[:, 16 * N:20 * N].rearrange("p (n c) -> p n c", c=4)
                nc.vector.tensor_add(S4, Q4, R4)
                # 17: shift evens <- odds (out even cols = state odd cols)
                nc.vector.tensor_mul(evens, odds,
                                     mkap(ones[:], 0, [[0, N], [0, 5]]))
                # 18: df2 slots (S odd cols 3,5,7,9) = -0.2*b4 + S4
                b4 = S5[:, :, 1:5, 1]
                nc.vector.scalar_tensor_tensor(b4, b4, -0.2, S4, mul, add)
                # 19: f2 (S col 1) = -0.2*v + P
                v3 = S5[:, :, 0:1, 1]
                nc.vector.scalar_tensor_tensor(v3, v3, -0.2, P3, mul, add)

                nc.sync.dma_start(ov[:, c * 10 * N:(c + 1) * 10 * N], S_t[:])

    orig = nc.to_json_bytes
    nc.to_json_bytes = lambda: _fix_bir_json(orig())
    return nc


V2_CHUNKS = 7


def _get_program(B: int, reps: int = 1):
    key = (B, reps)
    if key not in _CACHE:
        N = -(-B // (N_CORES * P * V2_CHUNKS))  # ceil
        R = P * V2_CHUNKS * N
        _CACHE[key] = (_build2(R, N, reps, V2_CHUNKS), R)
    return _CACHE[key]


def _run(state: np.ndarray, K: np.ndarray, trace: bool = False, reps: int = 1):
    from concourse import bass_utils

    B = state.shape[0]
    nc, R = _get_program(B, reps)
    BP = N_CORES * R

    state_p = np.zeros((BP, 10), dtype=np.float32)
    state_p[:B] = state
    k_p = np.zeros((BP, 4), dtype=np.float32)
    k_p[:B] = K

    in_maps = [
        {"state": state_p[i * R:(i + 1) * R], "K": k_p[i * R:(i + 1) * R]}
        for i in range(N_CORES)
    ]
    res = bass_utils.run_bass_kernel_spmd(
        nc, in_maps, core_ids=list(range(N_CORES)), trace=trace
    )
    out = np.concatenate([r["out"] for r in res.results], axis=0)[:B]
    return out, res


def kernel(t, state, K):
    state = np.ascontiguousarray(np.asarray(state), dtype=np.float32)
    K = np.ascontiguousarray(np.asarray(K), dtype=np.float32)
    out, _ = _run(state, K, trace=False)
    return out


# revision 12
# speedup vs baseline: 604.1361x; 80.2939x over previous
"""Trainium2 Bass kernel for the augmented-ODE-RHS (primal + 4 JVPs) problem.

Math (per sample; w=omega, v=omega_dot, K=(k0..k3), aug pairs (a_p, b_p)):
    mM = k0*w + k1*v            M = 10 - mM        A = 1/M
    mD = k2*w + k3*v            E = mD - 1 (= -D)
    u  = 0.2*w + v
    g  = 0.02 - 4*w + E*u       P = A*g
    f2 = P - 0.2*v
    out[0] = v, out[1] = f2
JVP p (tangent (a_p, b_p, e_p)) collapses to a per-sample linear form:
    alpha = -4A + 0.2*A*E + (A*u)*k2 + (A*P)*k0
    beta  = A*E - 0.2 + (A*u)*k3 + (A*P)*k1
    gamma_p in (A*P*w, A*P*v, A*u*w, A*u*v)
    out[2+2p] = b_p,  out[3+2p] = alpha*a_p + beta*b_p + gamma_p

Sharding: pure data parallel over the batch across 8 NeuronCores. Each core
gets R = 128*CHUNKS*N rows (inputs zero-padded up to 8R). Per core, rows are
laid out so SBUF partition j owns a contiguous slab of rows -> every DMA is
128 fully-contiguous multi-KB segments.

Engine split per chunk (fp32): DVE does the tensor*tensor products (30N),
GPSIMD the pure adds (14N), ACT the affine/copies (9N), HWDGE the DMAs.
"""

import json

import numpy as np

N_CORES = 8
P = 128
CHUNKS = 10

_CACHE: dict = {}


def _fix_bir_json(raw: bytes) -> bytes:
    """Walrus in this container encodes at most ONE sem-wait and ONE sem-update
    per instruction. Tile attaches several. Split the extras onto single-wait /
    single-update EventSemaphore instructions on the same engine, placed just
    before (waits) / after (updates) the original — identical sync semantics."""
    m = json.loads(raw)
    ctr = 0
    for fn in m["functions"]:
        for blk in fn["blocks"]:
            out = []
            for ins in blk["instructions"]:
                si = ins.get("sync_info")
                pend_updates = []
                if si:
                    waits = si.get("on_wait") or []
                    if len(waits) > 1:
                        for w in waits[:-1]:
                            ctr += 1
                            ev = {
                                "engine": ins["engine"], "ins": [], "outs": [],
                                "name": f"xw-{ctr}",
                                "opcode": "EventSemaphore",
                                "sync_info": {"on_update": [], "on_wait": [w]},
                            }
                            if "debug" in ins:
                                ev["debug"] = ins["debug"]
                            out.append(ev)
                        si["on_wait"] = [waits[-1]]
                    ups = si.get("on_update") or []
                    if len(ups) > 1:
                        assert ins.get("opcode") != "DMACopy", \
                            "DMACopy with >1 sem updates cannot be split"
                        si["on_update"] = [ups[0]]
                        pend_updates = ups[1:]
                out.append(ins)
                for u in pend_updates:
                    ctr += 1
                    ev = {
                        "engine": ins["engine"], "ins": [], "outs": [],
                        "name": f"xu-{ctr}",
                        "opcode": "EventSemaphore",
                        "sync_info": {"on_update": [u], "on_wait": []},
                    }
                    if "debug" in ins:
                        ev["debug"] = ins["debug"]
                    out.append(ev)
            blk["instructions"] = out
    return json.dumps(m).encode()


def _build(R: int, N: int, reps: int = 1):
    import concourse.bass as bass
    import concourse.tile as tile
    import concourse.mybir as mybir

    F32 = mybir.dt.float32
    mul = mybir.AluOpType.mult
    add = mybir.AluOpType.add
    Copy = mybir.ActivationFunctionType.Copy

    nc = bass.Bass("TRN2")

    state_d = nc.dram_tensor("state", [R, 10], F32, kind="ExternalInput")
    k_d = nc.dram_tensor("K", [R, 4], F32, kind="ExternalInput")
    out_d = nc.dram_tensor("out", [R, 10], F32, kind="ExternalOutput")

    sv = state_d[:].rearrange("(p n) m -> p (n m)", p=P)
    kv = k_d[:].rearrange("(p n) m -> p (n m)", p=P)
    ov = out_d[:].rearrange("(p n) m -> p (n m)", p=P)

    with tile.TileContext(nc) as tc:
        with (
            tc.tile_pool(name="io", bufs=2) as io,
            tc.tile_pool(name="tmp", bufs=1) as tp,
            tc.tile_pool(name="tmp2", bufs=2) as tp2,
        ):
            for c in [c for _ in range(reps) for c in range(CHUNKS)]:
                S_t = io.tile([P, 10 * N], F32, tag="S")
                K_t = io.tile([P, 4 * N], F32, tag="K")
                O_t = io.tile([P, 10 * N], F32, tag="O")
                nc.sync.dma_start(S_t[:], sv[:, c * 10 * N:(c + 1) * 10 * N])
                nc.sync.dma_start(K_t[:], kv[:, c * 4 * N:(c + 1) * 4 * N])

                S5 = S_t[:].rearrange("p (n c two) -> p n c two", two=2, c=5)
                O5 = O_t[:].rearrange("p (n c two) -> p n c two", two=2, c=5)
                Kt22 = K_t[:].rearrange("p (n c two) -> p n c two", two=2, c=2)
                Kt4 = K_t[:].rearrange("p (n f) -> p n f", f=4)

                w3 = S5[:, :, 0:1, 0]     # [P,N,1]
                v3 = S5[:, :, 0:1, 1]
                wv3 = S5[:, :, 0, :]      # [P,N,2]
                a4 = S5[:, :, 1:5, 0]     # [P,N,4]
                b4 = S5[:, :, 1:5, 1]
                k02 = Kt22[:, :, :, 0]    # (k0,k2)
                k13 = Kt22[:, :, :, 1]    # (k1,k3)
                k01 = Kt4[:, :, 0:2]
                k23 = Kt4[:, :, 2:4]

                X_t = tp.tile([P, 2 * N], F32, tag="X")
                Y_t = tp.tile([P, 2 * N], F32, tag="Y")
                MD_t = tp2.tile([P, 2 * N], F32, tag="MD")
                Mb_t = tp.tile([P, N], F32, tag="Mb")
                ln_t = tp.tile([P, N], F32, tag="ln")
                A_t = tp2.tile([P, N], F32, tag="A")
                E_t = tp2.tile([P, N], F32, tag="E")
                PU_t = tp2.tile([P, 2 * N], F32, tag="PU")
                T3_t = tp.tile([P, N], F32, tag="T3")
                h_t = tp.tile([P, N], F32, tag="h")
                AE_t = tp.tile([P, N], F32, tag="AE")
                CMU_t = tp2.tile([P, 2 * N], F32, tag="CMU")
                ca0_t = tp.tile([P, N], F32, tag="ca0")
                CAB_t = tp2.tile([P, 2 * N], F32, tag="CAB")
                T4_t = tp.tile([P, 2 * N], F32, tag="T4")
                T5_t = tp.tile([P, 2 * N], F32, tag="T5")
                T6_t = tp.tile([P, 2 * N], F32, tag="T6")
                AB_t = tp2.tile([P, 2 * N], F32, tag="AB")
                T7a_t = tp.tile([P, 4 * N], F32, tag="T7a")
                T7b_t = tp.tile([P, 4 * N], F32, tag="T7b")
                T8_t = tp.tile([P, 4 * N], F32, tag="T8")
                G_t = tp.tile([P, 4 * N], F32, tag="G")

                X2 = X_t[:].rearrange("p (n two) -> p n two", two=2)
                Y2 = Y_t[:].rearrange("p (n two) -> p n two", two=2)
                MD2 = MD_t[:].rearrange("p (n two) -> p n two", two=2)
                PU2 = PU_t[:].rearrange("p (n two) -> p n two", two=2)
                CMU2 = CMU_t[:].rearrange("p (n two) -> p n two", two=2)
                CAB2 = CAB_t[:].rearrange("p (n two) -> p n two", two=2)
                AB2 = AB_t[:].rearrange("p (n two) -> p n two", two=2)
                T7a2 = T7a_t[:].rearrange("p (n f) -> p n f", f=4)
                T7b2 = T7b_t[:].rearrange("p (n f) -> p n f", f=4)
                T82 = T8_t[:].rearrange("p (n f) -> p n f", f=4)
                G2 = G_t[:].rearrange("p (n f) -> p n f", f=4)

                A3 = A_t[:].unsqueeze(2)
                E3 = E_t[:].unsqueeze(2)

                # X=(k0,k2)*w ; Y=(k1,k3)*v ; MD=X+Y=(mM,mD)
                nc.vector.tensor_mul(X2, k02, w3.broadcast_to([P, N, 2]))
                nc.vector.tensor_mul(Y2, k13, v3.broadcast_to([P, N, 2]))
                nc.gpsimd.tensor_add(MD_t[:], X_t[:], Y_t[:])

                # Mb = 10 - mM ; E = mD - 1 ; A = 1/Mb
                nc.scalar.activation(Mb_t[:].unsqueeze(2), MD2[:, :, 0:1], Copy,
                                     bias=10.0, scale=-1.0)
                nc.scalar.activation(E3, MD2[:, :, 1:2], Copy,
                                     bias=-1.0, scale=1.0)
                # A = 1/Mb via exp(-ln(Mb)) on ACT (Mb > 0 always: Mb = 10 - mM)
                nc.scalar.activation(ln_t[:], Mb_t[:],
                                     mybir.ActivationFunctionType.Ln)
                nc.scalar.activation(A_t[:], ln_t[:],
                                     mybir.ActivationFunctionType.Exp, scale=-1.0)

                # u = 0.2w + v ; T3 = E*u ; h = -4w + T3 ; P = (h+0.02)*A
                nc.vector.scalar_tensor_tensor(PU2[:, :, 0:1], w3, 0.2, v3, mul, add)
                nc.vector.tensor_mul(T3_t[:].unsqueeze(2), E3, PU2[:, :, 0:1])
                nc.vector.scalar_tensor_tensor(h_t[:].unsqueeze(2), w3, -4.0,
                                               T3_t[:].unsqueeze(2), mul, add)
                nc.vector.scalar_tensor_tensor(PU2[:, :, 1:2], h_t[:].unsqueeze(2),
                                               0.02, A3, add, mul)

                # AE = A*E ; (c_u,c_m) = A*(u,P)
                nc.vector.tensor_mul(AE_t[:].unsqueeze(2), A3, E3)
                nc.vector.tensor_mul(CMU2, A3.broadcast_to([P, N, 2]), PU2)

                # c_a = 0.2AE - 4A ; c_b = AE - 0.2
                nc.scalar.activation(ca0_t[:].unsqueeze(2), A3, Copy, scale=-4.0)
                nc.vector.scalar_tensor_tensor(CAB2[:, :, 0:1], AE_t[:].unsqueeze(2),
                                               0.2, ca0_t[:].unsqueeze(2), mul, add)
                nc.scalar.activation(CAB2[:, :, 1:2], AE_t[:].unsqueeze(2), Copy,
                                     bias=-0.2, scale=1.0)

                c_u_bc2 = CMU2[:, :, 0:1].broadcast_to([P, N, 2])
                c_m_bc2 = CMU2[:, :, 1:2].broadcast_to([P, N, 2])

                # (alpha,beta) = (c_a,c_b) + c_u*(k2,k3) + c_m*(k0,k1)
                nc.vector.tensor_mul(
                    T4_t[:].rearrange("p (n two) -> p n two", two=2), c_u_bc2, k23)
                nc.vector.tensor_mul(
                    T5_t[:].rearrange("p (n two) -> p n two", two=2), c_m_bc2, k01)
                nc.gpsimd.tensor_add(T6_t[:], T4_t[:], T5_t[:])
                nc.gpsimd.tensor_add(AB_t[:], T6_t[:], CAB_t[:])

                # f2 = P - 0.2v -> out col 1
                nc.vector.scalar_tensor_tensor(O5[:, :, 0:1, 1], v3, -0.2,
                                               PU2[:, :, 1:2], mul, add)

                # df2_p = alpha*a_p + beta*b_p + gamma_p -> out cols 3,5,7,9
                nc.vector.tensor_mul(T7a2, AB2[:, :, 0:1].broadcast_to([P, N, 4]), a4)
                nc.vector.tensor_mul(T7b2, AB2[:, :, 1:2].broadcast_to([P, N, 4]), b4)
                nc.gpsimd.tensor_add(T8_t[:], T7a_t[:], T7b_t[:])
                nc.vector.tensor_mul(G2[:, :, 0:2], c_m_bc2, wv3)
                nc.vector.tensor_mul(G2[:, :, 2:4], c_u_bc2, wv3)
                nc.gpsimd.tensor_add(O5[:, :, 1:5, 1], T82, G2)

                # out even cols = state odd cols
                nc.scalar.activation(O5[:, :, :, 0], S5[:, :, :, 1], Copy)

                nc.sync.dma_start(ov[:, c * 10 * N:(c + 1) * 10 * N], O_t[:])

    orig = nc.to_json_bytes
    nc.to_json_bytes = lambda: _fix_bir_json(orig())
    return nc


def _build2(R: int, N: int, reps: int = 1, chunks: int = 7):
    """v2: single-engine (DVE-only) minimal-instruction design.

    This platform charges a large fixed cost per engine instruction, so the
    kernel is organised as ~18 wide DVE ops per chunk, no cross-engine sync
    (outputs are computed in-place in the input state tile), HWDGE DMAs.
    """
    import concourse.bass as bass
    import concourse.tile as tile
    import concourse.mybir as mybir
    from concourse.ap import AP

    F32 = mybir.dt.float32
    mul = mybir.AluOpType.mult
    add = mybir.AluOpType.add
    sub = mybir.AluOpType.subtract

    nc = bass.Bass("TRN2")
    state_d = nc.dram_tensor("state", [R, 10], F32, kind="ExternalInput")
    k_d = nc.dram_tensor("K", [R, 4], F32, kind="ExternalInput")
    out_d = nc.dram_tensor("out", [R, 10], F32, kind="ExternalOutput")
    sv = state_d[:].rearrange("(p n) m -> p (n m)", p=P)
    kv = k_d[:].rearrange("(p n) m -> p (n m)", p=P)
    ov = out_d[:].rearrange("(p n) m -> p (n m)", p=P)

    def mkap(tile_ap, offset, dims):
        # dims: list of [step, count] free dims; partition dim taken from tile
        part = tile_ap.ap[0]
        return AP(tile_ap.tensor, offset, [list(part)] + [list(d) for d in dims])

    with tile.TileContext(nc) as tc:
        with (
            tc.tile_pool(name="io", bufs=2) as io,
            tc.tile_pool(name="tmp", bufs=1) as tp,
            tc.tile_pool(name="const", bufs=1) as cp,
        ):
            C2 = cp.tile([P, 2], F32)      # [10, 1]
            ones = cp.tile([P, 1], F32)
            nc.vector.memset(C2[:, 0:1], 10.0)
            nc.vector.memset(C2[:, 1:2], 1.0)
            nc.vector.memset(ones[:], 1.0)

            for c in [c for _ in range(reps) for c in range(chunks)]:
                S_t = io.tile([P, 10 * N], F32, tag="S")
                K_t = io.tile([P, 4 * N], F32, tag="K")
                nc.sync.dma_start(S_t[:], sv[:, c * 10 * N:(c + 1) * 10 * N])
                nc.sync.dma_start(K_t[:], kv[:, c * 4 * N:(c + 1) * 4 * N])

                SC = tp.tile([P, 20 * N], F32, tag="SC")
                ZZ = tp.tile([P, 10 * N], F32, tag="ZZ")
                U5_t = tp.tile([P, 5 * N], F32, tag="U5")
                DU5_t = tp.tile([P, 5 * N], F32, tag="DU5")
                H5_t = tp.tile([P, 5 * N], F32, tag="H5")
                MD_t = tp.tile([P, 2 * N], F32, tag="MD")
                A_t = tp.tile([P, N], F32, tag="A")
                P_t = tp.tile([P, N], F32, tag="P")
                cm_t = tp.tile([P, N], F32, tag="cm")

                S5 = S_t[:].rearrange("p (n c two) -> p n c two", two=2, c=5)
                evens = S5[:, :, :, 0]                    # [P,N,5] strides (10,2)
                odds = S5[:, :, :, 1]
                ev_rep = evens.unsqueeze(2).broadcast_to([P, N, 2, 5])
                od_rep = odds.unsqueeze(2).broadcast_to([P, N, 2, 5])
                Kt22 = K_t[:].rearrange("p (n c two) -> p n c two", two=2, c=2)
                K02 = Kt22[:, :, :, 0].unsqueeze(3).broadcast_to([P, N, 2, 5])
                K13 = Kt22[:, :, :, 1].unsqueeze(3).broadcast_to([P, N, 2, 5])

                E2v = SC[:, :10 * N].rearrange("p (n a c) -> p n a c", a=2, c=5)
                Rv = SC[:, 10 * N:].rearrange("p (n a c) -> p n a c", a=2, c=5)
                ZZv = ZZ[:].rearrange("p (n a c) -> p n a c", a=2, c=5)
                U5v = U5_t[:].rearrange("p (n c) -> p n c", c=5)
                DU5v = DU5_t[:].rearrange("p (n c) -> p n c", c=5)
                H5v = H5_t[:].rearrange("p (n c) -> p n c", c=5)
                MDv = MD_t[:].rearrange("p (n c) -> p n c", c=2)
                A3 = A_t[:].unsqueeze(2)                  # [P,N,1]
                P3 = P_t[:].unsqueeze(2)
                cm3 = cm_t[:].unsqueeze(2)

                # 1-3: ZZ[j2,c] = k_{2j2}*S[2c] + k_{2j2+1}*S[2c+1]
                nc.vector.tensor_mul(E2v, K02, ev_rep)
                nc.vector.tensor_mul(Rv, K13, od_rep)
                nc.vector.tensor_add(ZZv, E2v, Rv)
                # 4: extras — ZZ slots {1,2,8,9} += (w,v,w,v)
                zz_ex = mkap(ZZ[:], 1, [[10, N], [7, 2], [1, 2]])
                wv_rep = mkap(S_t[:], 0, [[10, N], [0, 2], [1, 2]])
                nc.vector.tensor_add(zz_ex, zz_ex, wv_rep)
                # 5: MD = [10,1] - [mM, mD]
                c2b = mkap(C2[:], 0, [[0, N], [1, 2]])
                zz0 = mkap(ZZ[:], 0, [[10, N], [5, 2]])
                nc.vector.tensor_tensor(MDv, c2b, zz0, sub)
                # 6: A = 1/M
                nc.vector.reciprocal(A_t[:], MDv[:, :, 0])
                # 7: U5 = 0.2*evens + odds
                nc.vector.scalar_tensor_tensor(U5v, evens, 0.2, odds, mul, add)
                # 8: DU5 = D * U5
                nc.vector.tensor_mul(DU5v, MDv[:, :, 1:2].broadcast_to([P, N, 5]), U5v)
                # 9: NDU = u * nD_p   (SC[0:4N])
                NDU = SC[:, :4 * N].rearrange("p (n c) -> p n c", c=4)
                nc.vector.tensor_mul(NDU, U5v[:, :, 0:1].broadcast_to([P, N, 4]),
                                     ZZv[:, :, 1, 1:5])
                # 10: H5 = -4*evens - DU5
                nc.vector.scalar_tensor_tensor(H5v, evens, -4.0, DU5v, mul, sub)
                # 11: DG4 = H5[1:5] + NDU   (SC[4N:8N])
                DG4 = SC[:, 4 * N:8 * N].rearrange("p (n c) -> p n c", c=4)
                nc.vector.tensor_add(DG4, H5v[:, :, 1:5], NDU)
                # 12: P = (H5[0] + 0.02) * A
                nc.vector.scalar_tensor_tensor(P3, H5v[:, :, 0:1], 0.02, A3, add, mul)
                # 13: cm = A * P
                nc.vector.tensor_mul(cm3, A3, P3)
                # 14: Q4 = A * DG4   (SC[8N:12N])
                Q4 = SC[:, 8 * N:12 * N].rearrange("p (n c) -> p n c", c=4)
                nc.vector.tensor_mul(Q4, A3.broadcast_to([P, N, 4]), DG4)
                # 15: R4 = cm * nM_p   (SC[12N:16N])
                R4 = SC[:, 12 * N:16 * N].rearrange("p (n c) -> p n c", c=4)
                nc.vector.tensor_mul(R4, cm3.broadcast_to([P, N, 4]),
                                     ZZv[:, :, 0, 1:5])
                # 16: S4 = Q4 + R4   (SC[16N:20N])
                S4 = SC[:, 16 * N:20 * N].rearrange("p (n c) -> p n c", c=4)
                nc.vector.tensor_add(S4, Q4, R4)
                # 17: shift evens <- odds (out even cols = state odd cols)
                nc.vector.tensor_mul(evens, odds,
                                     mkap(ones[:], 0, [[0, N], [0, 5]]))
                # 18: df2 slots (S odd cols 3,5,7,9) = -0.2*b4 + S4
                b4 = S5[:, :, 1:5, 1]
                nc.vector.scalar_tensor_tensor(b4, b4, -0.2, S4, mul, add)
                # 19: f2 (S col 1) = -0.2*v + P
                v3 = S5[:, :, 0:1, 1]
                nc.vector.scalar_tensor_tensor(v3, v3, -0.2, P3, mul, add)

                nc.sync.dma_start(ov[:, c * 10 * N:(c + 1) * 10 * N], S_t[:])

    orig = nc.to_json_bytes
    nc.to_json_bytes = lambda: _fix_bir_json(orig())
    return nc


def _build3(R: int, N: int, reps: int = 1, chunks: int = 6):
    """v3: v2's math inside a hardware For_i loop over chunks.

    On this platform, first-time instruction streaming costs ~20-100us per
    instruction, but loop iterations re-execute from IRAM at normal speed —
    so the chunk pipeline is emitted once and looped with dynamic DMA
    offsets."""
    import concourse.bass as bass
    import concourse.tile as tile
    import concourse.mybir as mybir
    from concourse.ap import AP

    F32 = mybir.dt.float32
    mul = mybir.AluOpType.mult
    add = mybir.AluOpType.add
    sub = mybir.AluOpType.subtract

    nc = bass.Bass("TRN2")
    state_d = nc.dram_tensor("state", [R, 10], F32, kind="ExternalInput")
    k_d = nc.dram_tensor("K", [R, 4], F32, kind="ExternalInput")
    out_d = nc.dram_tensor("out", [R, 10], F32, kind="ExternalOutput")
    sv = state_d[:].rearrange("(p n) m -> p (n m)", p=P)
    kv = k_d[:].rearrange("(p n) m -> p (n m)", p=P)
    ov = out_d[:].rearrange("(p n) m -> p (n m)", p=P)

    def mkap(tile_ap, offset, dims):
        part = tile_ap.ap[0]
        return AP(tile_ap.tensor, offset, [list(part)] + [list(d) for d in dims])

    with tile.TileContext(nc) as tc:
        with (
            tc.tile_pool(name="io", bufs=1) as io,
            tc.tile_pool(name="tmp", bufs=1) as tp,
            tc.tile_pool(name="const", bufs=1) as cp,
        ):
            C2 = cp.tile([P, 2], F32)
            ones = cp.tile([P, 1], F32)
            nc.vector.memset(C2[:, 0:1], 10.0)
            nc.vector.memset(C2[:, 1:2], 1.0)
            nc.vector.memset(ones[:], 1.0)

            with tc.For_i(0, chunks * reps, 1) as iv:
                off = iv if reps == 1 else iv * 0

                S_t = io.tile([P, 10 * N], F32, tag="S")
                K_t = io.tile([P, 4 * N], F32, tag="K")
                nc.sync.dma_start(S_t[:], sv[:, bass.ts(off, 10 * N)])
                nc.sync.dma_start(K_t[:], kv[:, bass.ts(off, 4 * N)])

                SC = tp.tile([P, 20 * N], F32, tag="SC")
                ZZ = tp.tile([P, 10 * N], F32, tag="ZZ")
                U5_t = tp.tile([P, 5 * N], F32, tag="U5")
                DU5_t = tp.tile([P, 5 * N], F32, tag="DU5")
                H5_t = tp.tile([P, 5 * N], F32, tag="H5")
                MD_t = tp.tile([P, 2 * N], F32, tag="MD")
                A_t = tp.tile([P, N], F32, tag="A")
                P_t = tp.tile([P, N], F32, tag="P")
                cm_t = tp.tile([P, N], F32, tag="cm")

                S5 = S_t[:].rearrange("p (n c two) -> p n c two", two=2, c=5)
                evens = S5[:, :, :, 0]
                odds = S5[:, :, :, 1]
                ev_rep = evens.unsqueeze(2).broadcast_to([P, N, 2, 5])
                od_rep = odds.unsqueeze(2).broadcast_to([P, N, 2, 5])
                Kt22 = K_t[:].rearrange("p (n c two) -> p n c two", two=2, c=2)
                K02 = Kt22[:, :, :, 0].unsqueeze(3).broadcast_to([P, N, 2, 5])
                K13 = Kt22[:, :, :, 1].unsqueeze(3).broadcast_to([P, N, 2, 5])

                E2v = SC[:, :10 * N].rearrange("p (n a c) -> p n a c", a=2, c=5)
                Rv = SC[:, 10 * N:].rearrange("p (n a c) -> p n a c", a=2, c=5)
                ZZv = ZZ[:].rearrange("p (n a c) -> p n a c", a=2, c=5)
                U5v = U5_t[:].rearrange("p (n c) -> p n c", c=5)
                DU5v = DU5_t[:].rearrange("p (n c) -> p n c", c=5)
                H5v = H5_t[:].rearrange("p (n c) -> p n c", c=5)
                MDv = MD_t[:].rearrange("p (n c) -> p n c", c=2)
                A3 = A_t[:].unsqueeze(2)
                P3 = P_t[:].unsqueeze(2)
                cm3 = cm_t[:].unsqueeze(2)

                nc.vector.tensor_mul(E2v, K02, ev_rep)
                nc.vector.tensor_mul(Rv, K13, od_rep)
                nc.vector.tensor_add(ZZv, E2v, Rv)
                zz_ex = mkap(ZZ[:], 1, [[10, N], [7, 2], [1, 2]])
                wv_rep = mkap(S_t[:], 0, [[10, N], [0, 2], [1, 2]])
                nc.vector.tensor_add(zz_ex, zz_ex, wv_rep)
                c2b = mkap(C2[:], 0, [[0, N], [1, 2]])
                zz0 = mkap(ZZ[:], 0, [[10, N], [5, 2]])
                nc.vector.tensor_tensor(MDv, c2b, zz0, sub)
                nc.vector.reciprocal(A_t[:], MDv[:, :, 0])
                nc.vector.scalar_tensor_tensor(U5v, evens, 0.2, odds, mul, add)
                nc.vector.tensor_mul(DU5v, MDv[:, :, 1:2].broadcast_to([P, N, 5]),
                                     U5v)
                NDU = SC[:, :4 * N].rearrange("p (n c) -> p n c", c=4)
                nc.vector.tensor_mul(NDU, U5v[:, :, 0:1].broadcast_to([P, N, 4]),
                                     ZZv[:, :, 1, 1:5])
                nc.vector.scalar_tensor_tensor(H5v, evens, -4.0, DU5v, mul, sub)
                DG4 = SC[:, 4 * N:8 * N].rearrange("p (n c) -> p n c", c=4)
                nc.vector.tensor_add(DG4, H5v[:, :, 1:5], NDU)
                nc.vector.scalar_tensor_tensor(P3, H5v[:, :, 0:1], 0.02, A3,
                                               add, mul)
                nc.vector.tensor_mul(cm3, A3, P3)
                Q4 = SC[:, 8 * N:12 * N].rearrange("p (n c) -> p n c", c=4)
                nc.vector.tensor_mul(Q4, A3.broadcast_to([P, N, 4]), DG4)
                R4 = SC[:, 12 * N:16 * N].rearrange("p (n c) -> p n c", c=4)
                nc.vector.tensor_mul(R4, cm3.broadcast_to([P, N, 4]),
                                     ZZv[:, :, 0, 1:5])
                S4 = SC[:, 16 * N:20 * N].rearrange("p (n c) -> p n c", c=4)
                nc.vector.tensor_add(S4, Q4, R4)
                nc.vector.tensor_mul(evens, odds,
                                     mkap(ones[:], 0, [[0, N], [0, 5]]))
                b4 = S5[:, :, 1:5, 1]
                nc.vector.scalar_tensor_tensor(b4, b4, -0.2, S4, mul, add)
                v3 = S5[:, :, 0:1, 1]
                nc.vector.scalar_tensor_tensor(v3, v3, -0.2, P3, mul, add)

                nc.sync.dma_start(ov[:, bass.ts(off, 10 * N)], S_t[:])

    orig = nc.to_json_bytes
    nc.to_json_bytes = lambda: _fix_bir_json(orig())
    return nc


V3_CHUNKS = 6


def _get_program(B: int, reps: int = 1):
    key = (B, reps)
    if key not in _CACHE:
        N = -(-B // (N_CORES * P * V3_CHUNKS))  # ceil
        R = P * V3_CHUNKS * N
        _CACHE[key] = (_build3(R, N, reps, V3_CHUNKS), R)
    return _CACHE[key]


def _run(state: np.ndarray, K: np.ndarray, trace: bool = False, reps: int = 1):
    from concourse import bass_utils

    B = state.shape[0]
    nc, R = _get_program(B, reps)
    BP = N_CORES * R

    state_p = np.zeros((BP, 10), dtype=np.float32)
    state_p[:B] = state
    k_p = np.zeros((BP, 4), dtype=np.float32)
    k_p[:B] = K

    in_maps = [
        {"state": state_p[i * R:(i + 1) * R], "K": k_p[i * R:(i + 1) * R]}
        for i in range(N_CORES)
    ]
    res = bass_utils.run_bass_kernel_spmd(
        nc, in_maps, core_ids=list(range(N_CORES)), trace=trace
    )
    out = np.concatenate([r["out"] for r in res.results], axis=0)[:B]
    return out, res


def kernel(t, state, K):
    state = np.ascontiguousarray(np.asarray(state), dtype=np.float32)
    K = np.ascontiguousarray(np.asarray(K), dtype=np.float32)
    out, _ = _run(state, K, trace=False)
    return out
